# revision 28
# baseline (speedup 1.0000x reference)
"""Trainium2 Bass kernel for additive-attention pooling (v2).

Computation (per batch row b):
    Wah   = h @ Wah_w.T                         [B, HID]
    e     = tanh(Wah[:, None, :] + p_att_feats) [B, L, HID]
    s     = e @ alpha_w[0]                      [B, L]
    alpha = softmax(s, -1)                      [B, L]
    att   = sum_l alpha[b, l] * att_feats[b, l, :]   [B, FEAT]

Sharding: pure data parallel over the batch dim, 32 rows per core on 8
NeuronCores; the small Wah_w / alpha_w weights are replicated.

v2 dataflow changes vs the first working kernel (264 us):
  * p_att_feats arrives host-TRANSPOSED ([h%128 partitions, pair, hc,
    jb, l] bf16) so the kernel does ZERO PE transposes (the old kernel
    spent ~2/3 of its TensorE time on transpose matmuls and ran the PE
    at half clock from HAM oscillation).
  * The Wah broadcast-add moves off the ACT bias path onto DVE+GpSimd
    tensor_scalar adds (8 small adds per pair, engines alternated),
    followed by ONE tanh activation over the whole [128, 4*2*196] pair
    tile -- ACT instruction count drops 128 -> 16 for the tanh work.
  * Phase 2 (att = alpha^T @ att_feats) runs as 4-way column-tiled
    matmuls: batch j of a quad occupies PE column-group j
    (tile_position=(0,32j), alpha column as the stationary operand), so
    4 batches stream their att_feats concurrently and the PSUM output
    lands on partitions {0,32,64,96} -- the PSUM->SBUF copies are
    [4, 512] (4 active lanes) instead of [1, 512] (1 lane), which
    removes the ~80 us of single-lane copies the old kernel paid.
  * Outputs stage per quad and DMA out via strided-partition APs.

All data stays bf16 on the wire (fp8 was measured: rel_norm 2.7e-2 on
att_feats -- too close to the 2e-2 gate).

The walrus build in this image accepts only one semaphore wait and one
update per instruction; _split_sync() post-processes the scheduled BIR
to spread Tile's multi-wait/multi-update sync info onto NoOp carriers.
"""

import os
import sys
import types

sys.path.insert(0, "/opt/trn_rl_repo")

# This image's antenv package lacks axon_hooks; provide it so
# concourse.bass_utils can import it (trace path) without crashing.
if "antenv.axon_hooks" not in sys.modules:
    _m = types.ModuleType("antenv.axon_hooks")

    def _set_hook(h):
        _m._hook = h

    def _get_hook():
        return getattr(_m, "_hook", None)

    _m.set_axon_ntff_profile_hook = _set_hook
    _m.get_axon_ntff_profile_hook = _get_hook
    sys.modules["antenv.axon_hooks"] = _m
    import antenv

    antenv.axon_hooks = _m

import numpy as np  # noqa: E402
import bass_rust  # noqa: E402
import concourse.bass as bass  # noqa: E402
import concourse.tile as tile  # noqa: E402
from concourse import mybir  # noqa: E402

F32 = mybir.dt.float32
BF16 = mybir.dt.bfloat16
PSUM = bass.MemorySpace.PSUM
Tanh = mybir.ActivationFunctionType.Tanh
Exp = mybir.ActivationFunctionType.Exp

B, L, RNN, HID, FEAT = 256, 196, 1024, 512, 2048
NCORES = 8
BL = B // NCORES  # batch rows per core (32)
L_HI = 128
L_LO = L - L_HI  # 68
NHC = HID // 128  # 4 h chunks
NRC = RNN // 128  # 8 r chunks
NFQ = FEAT // 512  # 4 psum-bank-sized f chunks
NPAIR = BL // 2  # 16
NQUAD = BL // 4  # 8

AF_BUFS = int(os.environ.get("KERNEL_AF_BUFS", "2"))


def _split_sync(nc):
    """walrus in this image encodes at most ONE semaphore wait and ONE
    semaphore update per instruction; Tile freely emits several. Move the
    extras onto single-wait/single-update NoOp carriers on the same engine
    (engine queues are strict FIFO, so a preceding NoOp's wait gates the
    instruction and a following NoOp's update fires after it completes)."""
    dma_types = {
        "InstDMACopy",
        "InstTensorLoad",
        "InstTensorSave",
        "InstDmaTransposeAnt",
        "InstTensorCopy",
    }
    for f in nc.m.functions:
        for bb in f.blocks:
            new = []
            changed = False
            for ins in bb.instructions:
                si = ins.sync_info
                if si is None:
                    new.append(ins)
                    continue
                waits = list(si.on_wait)
                updates = list(si.on_update)
                if len(waits) <= 1 and len(updates) <= 1:
                    new.append(ins)
                    continue
                changed = True
                tname = type(ins).__name__
                for j, w in enumerate(waits[:-1]):
                    nop = mybir.InstNoOp(name=f"{ins.name}_w{j}", ins=[], outs=[])
                    nop.engine = ins.engine
                    nop.sync_info = bass_rust.SyncInfo(on_wait=[w], on_update=[])
                    new.append(nop)
                keep_w = waits[-1:]
                post_u = []
                keep_u = updates
                if len(updates) > 1:
                    if tname in dma_types:
                        raise RuntimeError(
                            f"DMA instruction {ins.name} carries {len(updates)} "
                            "sem updates; cannot split without changing semantics"
                        )
                    keep_u = updates[:1]
                    post_u = updates[1:]
                ins.sync_info = bass_rust.SyncInfo(on_wait=keep_w, on_update=keep_u)
                new.append(ins)
                for j, u in enumerate(post_u):
                    nop = mybir.InstNoOp(name=f"{ins.name}_u{j}", ins=[], outs=[])
                    nop.engine = ins.engine
                    nop.sync_info = bass_rust.SyncInfo(on_wait=[], on_update=[u])
                    new.append(nop)
            if changed:
                bb.instructions = new


def build_nc(split=True):
    """Inputs arrive host-packed (see _make_in_maps):
      h:       [RNN, BL]                 bf16  (r-major)
      Wah_w:   [RNN, HID]                bf16  (r-major)
      alpha_w: [1, HID]                  f32
      p_att_feats: [128, NPAIR, NHC, 2, L]     bf16 (h%128 on partitions)
      att_hi:  [NQUAD, 128, 4, FEAT]     bf16  (l rows 0..127)
      att_lo:  [NQUAD, L_LO, 4, FEAT]    bf16  (l rows 128..195)
    Output:
      out:     [4, NQUAD, NFQ, 512]      f32   (att[4*qd+j, 512*q+x] =
                                                out[j, qd, q, x])
    """
    nc = bass.Bass()
    h_d = nc.declare_dram_parameter("h", [RNN, BL], BF16, isOutput=False)
    pa_d = nc.declare_dram_parameter(
        "p_att_feats", [128, NPAIR, NHC, 2, L], BF16, isOutput=False
    )
    hi_d = nc.declare_dram_parameter(
        "att_hi", [NQUAD, 128, 4, FEAT], BF16, isOutput=False
    )
    lo_d = nc.declare_dram_parameter(
        "att_lo", [NQUAD, L_LO, 4, FEAT], BF16, isOutput=False
    )
    ww_d = nc.declare_dram_parameter("Wah_w", [RNN, HID], BF16, isOutput=False)
    aw_d = nc.declare_dram_parameter("alpha_w", [1, HID], F32, isOutput=False)
    out_d = nc.declare_dram_parameter("out", [4, NQUAD, NFQ, 512], BF16, isOutput=True)

    with tile.TileContext(nc) as tc:
        with tc.tile_pool(name="singles", bufs=1) as singles:
            wahT = singles.tile([128, NHC, BL], F32)  # WahT[h % 128, hc, b]
            awT = singles.tile([128, NHC], BF16)  # alpha_w^T chunks
            # whole-core p_att stage (bf16, h on partitions): 50KB/partition
            pa_all = singles.tile([128, NPAIR, NHC, 2, L], BF16)
            # exp(scores), 256-wide zero-padded slot per batch so the lo
            # alphaT transpose matmul can span a full 128 output partitions
            expS = singles.tile([1, BL, 256], BF16)
            nc.gpsimd.memset(expS[:], 0.0)
            sums = singles.tile([1, BL], F32)
            rsum = singles.tile([1, BL], F32)
            # 1/sum replicated 32-wide so the alphaT transpose matmuls can
            # produce alpha replicated across 32 columns (-> M=32 phase-2
            # weights that write every partition of their PSUM col group)
            rsum_rep = singles.tile([1, BL, 32], BF16)
            ones_row = singles.tile([1, 32], BF16)
            nc.gpsimd.memset(ones_row[:], 1.0)
            aT_sb = singles.tile([128, BL, 2, 32], BF16)  # alphaT cols (hi, lo)
            # whole-core output stage (bf16): partition 32j holds the att
            # rows of quad-batch j (the other partitions carry copy junk
            # that the output DMAs never read)
            osb_all = singles.tile([128, NQUAD, NFQ, 512], BF16)

            # Batch-loop SBUF pools are allocated FIRST so their zones never
            # overlap the setup pool's -- otherwise the first input DMAs
            # inherit released-zone deps on the setup computation.
            with (
                tc.tile_pool(name="hi", bufs=AF_BUFS) as pool_hi,
                tc.tile_pool(name="lo", bufs=AF_BUFS) as pool_lo,
                tc.tile_pool(name="ea", bufs=3) as pool_ea,
                tc.tile_pool(name="e", bufs=3) as pool_e,
            ):
                # ---------------- setup: weights ----------------
                with (
                    tc.tile_pool(name="setup_sb", bufs=1) as ssb,
                    tc.tile_pool(name="setup_ps", bufs=2, space=PSUM) as sps,
                    tc.tile_pool(name="setup_acc", bufs=1, space=PSUM) as sacc,
                ):
                    hT = ssb.tile([128, NRC, BL], BF16)
                    nc.sync.dma_start(
                        hT[:], h_d[:].rearrange("(rc p) b -> p rc b", p=128)
                    )
                    wwT = ssb.tile([128, NRC, HID], BF16)
                    nc.sync.dma_start(
                        wwT[:], ww_d[:].rearrange("(rc p) c -> p rc c", p=128)
                    )
                    aw_sb = ssb.tile([1, HID], F32)
                    nc.sync.dma_start(aw_sb[:], aw_d[:])
                    ones11 = ssb.tile([1, 1], F32)
                    nc.gpsimd.memset(ones11[:], 1.0)

                    # alpha_w^T columns (bf16 to match bf16 e tiles)
                    for hc in range(NHC):
                        ps = sps.tile([128, 1], F32, tag="aw")
                        nc.tensor.matmul(
                            ps[:],
                            aw_sb[0:1, hc * 128 : (hc + 1) * 128],
                            ones11[:],
                            start=True,
                            stop=True,
                        )
                        nc.vector.tensor_copy(awT[:, hc : hc + 1], ps[:])

                    # WahT[h, b] = sum_r Wah_w[h, r] * h[b, r]
                    wahT_ps = [
                        sacc.tile([128, BL], F32, tag=f"acc{hc}", name=f"wahT_ps{hc}")
                        for hc in range(NHC)
                    ]
                    for rc in range(NRC):
                        for hc in range(NHC):
                            nc.tensor.matmul(
                                wahT_ps[hc][:],
                                wwT[:, rc, hc * 128 : (hc + 1) * 128],
                                hT[:, rc, :],
                                start=(rc == 0),
                                stop=(rc == NRC - 1),
                            )
                    for hc in range(NHC):
                        nc.vector.tensor_copy(wahT[:, hc, :], wahT_ps[hc][:])

                # p_att stream: 4 chunk DMAs into the whole-core stage, on
                # the SWDGE (gpsimd) queue so they don't contend with the
                # att_feats stream on the HWDGE rings
                for c in range(4):
                    nc.gpsimd.dma_start(
                        pa_all[:, 4 * c : 4 * c + 4], pa_d[:, 4 * c : 4 * c + 4]
                    )

                # ---------------- streaming batch loop ----------------
                with (
                    tc.tile_pool(name="sc_ps", bufs=2, space=PSUM) as pool_sc,
                    tc.tile_pool(name="aT_ps", bufs=2, space=PSUM) as pool_aT,
                    tc.tile_pool(name="ao_ps", bufs=1, space=PSUM) as pool_ao,
                ):
                    def phase2(quad, af_hi, af_lo):
                        for q in range(NFQ):
                            ao = pool_ao.tile([128, 512], F32, tag=f"q{q}")
                            fsl = slice(q * 512, (q + 1) * 512)
                            for j in range(4):
                                nc.tensor.matmul(
                                    ao[32 * j : 32 * j + 32, :],
                                    aT_sb[:, 4 * quad + j, 0],
                                    af_hi[:, j, fsl],
                                    start=True,
                                    stop=False,
                                    tile_position=(0, 32 * j),
                                )
                                nc.tensor.matmul(
                                    ao[32 * j : 32 * j + 32, :],
                                    aT_sb[0:L_LO, 4 * quad + j, 1],
                                    af_lo[0:L_LO, j, fsl],
                                    start=False,
                                    stop=True,
                                    tile_position=(0, 32 * j),
                                )
                            # full-width copy: partition-strided APs are
                            # illegal on compute engines; copying all 128
                            # lanes costs the same (per-lane elements).
                            # Split across DVE and ACT to balance load.
                            if q % 2 == 0:
                                nc.vector.tensor_copy(
                                    osb_all[:, quad, q, :], ao[:]
                                )
                            else:
                                nc.scalar.copy(osb_all[:, quad, q, :], ao[:])

                    # Software-pipelined: phase 2 of quad q-1 is emitted
                    # AFTER phase 1 of quad q, so the PE queue never
                    # head-of-line blocks on the af DMA of the current quad
                    # (the DMA issued a full quad-period before phase2
                    # consumes it).
                    prev = None
                    for quad in range(NQUAD):
                        # hi on the SP HWDGE ring, lo on the ACT HWDGE ring:
                        # two queues keep more SDMA descriptors in flight
                        # (one queue measured only 143 GB/s on this stream)
                        af_hi = pool_hi.tile([128, 4, FEAT], BF16, tag="hi")
                        nc.sync.dma_start(af_hi[:], hi_d[quad])
                        af_lo = pool_lo.tile([L_LO, 4, FEAT], BF16, tag="lo")
                        nc.scalar.dma_start(af_lo[:], lo_d[quad])

                        # ---- phase 1 for the quad's two pairs ----
                        for pp in range(2):
                            p = 2 * quad + pp
                            b0 = 2 * p
                            # Wah broadcast-adds on DVE (265ns each true
                            # cost; GpSimd's generic path is 3.6us -- keep
                            # it off), then ONE tanh over the whole pair
                            # tile so ACT pays the 352-cycle instruction
                            # overhead once instead of 8 times
                            ea = pool_ea.tile([128, NHC, 2, L], BF16)
                            for hc in range(NHC):
                                for jb in range(2):
                                    b = b0 + jb
                                    nc.vector.tensor_scalar_add(
                                        ea[:, hc, jb, :],
                                        pa_all[:, p, hc, jb, :],
                                        wahT[:, hc, b : b + 1],
                                    )
                            e = pool_e.tile([128, NHC, 2, L], BF16)
                            nc.scalar.activation(e[:], ea[:], Tanh)

                            sc = pool_sc.tile([1, 2, L], F32)
                            for hc in range(NHC):
                                nc.tensor.matmul(
                                    sc[:],
                                    awT[:, hc : hc + 1],
                                    e[:, hc],
                                    start=(hc == 0),
                                    stop=(hc == NHC - 1),
                                )

                            for jb in range(2):
                                b = b0 + jb
                                nc.scalar.activation(
                                    expS[0:1, b, 0:L],
                                    sc[0:1, jb, :],
                                    Exp,
                                    accum_out=sums[0:1, b : b + 1],
                                )
                                nc.vector.reciprocal(
                                    rsum[0:1, b : b + 1], sums[0:1, b : b + 1]
                                )
                                nc.vector.tensor_scalar_mul(
                                    rsum_rep[0:1, b, :],
                                    ones_row[:],
                                    rsum[0:1, b : b + 1],
                                )
                                # alphaT columns via K=1 matmuls; rhs = the
                                # replicated 1/sum row, folding the softmax
                                # normalization in and replicating alpha to
                                # 32 columns
                                aT = pool_aT.tile([128, 2, 32], F32)
                                nc.tensor.matmul(
                                    aT[:, 0, :],
                                    expS[0:1, b, 0:128],
                                    rsum_rep[0:1, b, :],
                                    start=True,
                                    stop=True,
                                )
                                nc.tensor.matmul(
                                    aT[:, 1, :],
                                    expS[0:1, b, 128:256],
                                    rsum_rep[0:1, b, :],
                                    start=True,
                                    stop=True,
                                )
                                nc.vector.tensor_copy(aT_sb[:, b], aT[:])

                        # ---- phase 2 for the PREVIOUS quad ----
                        if prev is not None:
                            phase2(*prev)
                        prev = (quad, af_hi, af_lo)
                    phase2(*prev)

                    # final output DMAs: partition 32j carries quad-batch j
                    for j in range(4):
                        nc.sync.dma_start(
                            out_d[j : j + 1], osb_all[32 * j : 32 * j + 1]
                        )

    if split:
        _split_sync(nc)
    return nc


_NC_CACHE = None


def _get_nc():
    global _NC_CACHE
    if _NC_CACHE is None:
        _NC_CACHE = build_nc()
    return _NC_CACHE


def _make_in_maps(h, att_feats, p_att_feats, Wah_w, alpha_w):
    import ml_dtypes

    bf = ml_dtypes.bfloat16
    h = np.ascontiguousarray(h, dtype=np.float32)
    att_feats = np.ascontiguousarray(att_feats, dtype=np.float32)
    p_att_feats = np.ascontiguousarray(p_att_feats, dtype=np.float32)
    wwT_host = np.ascontiguousarray(Wah_w.T).astype(bf)  # [RNN, HID]
    alpha_w = np.ascontiguousarray(alpha_w, dtype=np.float32)
    in_maps = []
    for i in range(NCORES):
        sl = slice(i * BL, (i + 1) * BL)
        # p_att: [BL, L, HID] -> [128, NPAIR, NHC, 2, L]
        pa = (
            p_att_feats[sl]
            .reshape(NPAIR, 2, L, NHC, 128)
            .transpose(4, 0, 3, 1, 2)
            .astype(bf)
        )
        af = att_feats[sl].reshape(NQUAD, 4, L, FEAT)
        af_hi = np.ascontiguousarray(af[:, :, :L_HI].transpose(0, 2, 1, 3)).astype(bf)
        af_lo = np.ascontiguousarray(af[:, :, L_HI:].transpose(0, 2, 1, 3)).astype(bf)
        in_maps.append(
            {
                "h": np.ascontiguousarray(h[sl].T).astype(bf),
                "p_att_feats": np.ascontiguousarray(pa),
                "att_hi": af_hi,
                "att_lo": af_lo,
                "Wah_w": wwT_host,
                "alpha_w": alpha_w,
            }
        )
    return in_maps


def _unpack_out(o):
    """[4, NQUAD, NFQ, 512] -> [BL, FEAT]"""
    return np.ascontiguousarray(
        np.asarray(o, dtype=np.float32).transpose(1, 0, 2, 3).reshape(BL, FEAT)
    )


def run_spmd(h, att_feats, p_att_feats, Wah_w, alpha_w, trace=False):
    """Run the SPMD kernel; returns (full_output, BassKernelResults)."""
    from concourse.bass_utils import run_bass_kernel_spmd

    nc = _get_nc()
    in_maps = _make_in_maps(h, att_feats, p_att_feats, Wah_w, alpha_w)
    res = run_bass_kernel_spmd(nc, in_maps, list(range(NCORES)), trace=trace)
    out = np.concatenate(
        [_unpack_out(res.results[i]["out"]) for i in range(NCORES)], axis=0
    )
    return out, res


def kernel(h, att_feats, p_att_feats, Wah_w, alpha_w):
    out, _ = run_spmd(h, att_feats, p_att_feats, Wah_w, alpha_w, trace=False)
    return out


# revision 29
# speedup vs baseline: 1.0080x; 1.0080x over previous
"""Trainium2 Bass kernel for additive-attention pooling (v2).

Computation (per batch row b):
    Wah   = h @ Wah_w.T                         [B, HID]
    e     = tanh(Wah[:, None, :] + p_att_feats) [B, L, HID]
    s     = e @ alpha_w[0]                      [B, L]
    alpha = softmax(s, -1)                      [B, L]
    att   = sum_l alpha[b, l] * att_feats[b, l, :]   [B, FEAT]

Sharding: pure data parallel over the batch dim, 32 rows per core on 8
NeuronCores; the small Wah_w / alpha_w weights are replicated.

v2 dataflow changes vs the first working kernel (264 us):
  * p_att_feats arrives host-TRANSPOSED ([h%128 partitions, pair, hc,
    jb, l] bf16) so the kernel does ZERO PE transposes (the old kernel
    spent ~2/3 of its TensorE time on transpose matmuls and ran the PE
    at half clock from HAM oscillation).
  * The Wah broadcast-add moves off the ACT bias path onto DVE+GpSimd
    tensor_scalar adds (8 small adds per pair, engines alternated),
    followed by ONE tanh activation over the whole [128, 4*2*196] pair
    tile -- ACT instruction count drops 128 -> 16 for the tanh work.
  * Phase 2 (att = alpha^T @ att_feats) runs as 4-way column-tiled
    matmuls: batch j of a quad occupies PE column-group j
    (tile_position=(0,32j), alpha column as the stationary operand), so
    4 batches stream their att_feats concurrently and the PSUM output
    lands on partitions {0,32,64,96} -- the PSUM->SBUF copies are
    [4, 512] (4 active lanes) instead of [1, 512] (1 lane), which
    removes the ~80 us of single-lane copies the old kernel paid.
  * Outputs stage per quad and DMA out via strided-partition APs.

All data stays bf16 on the wire (fp8 was measured: rel_norm 2.7e-2 on
att_feats -- too close to the 2e-2 gate).

The walrus build in this image accepts only one semaphore wait and one
update per instruction; _split_sync() post-processes the scheduled BIR
to spread Tile's multi-wait/multi-update sync info onto NoOp carriers.
"""

import os
import sys
import types

sys.path.insert(0, "/opt/trn_rl_repo")

# This image's antenv package lacks axon_hooks; provide it so
# concourse.bass_utils can import it (trace path) without crashing.
if "antenv.axon_hooks" not in sys.modules:
    _m = types.ModuleType("antenv.axon_hooks")

    def _set_hook(h):
        _m._hook = h

    def _get_hook():
        return getattr(_m, "_hook", None)

    _m.set_axon_ntff_profile_hook = _set_hook
    _m.get_axon_ntff_profile_hook = _get_hook
    sys.modules["antenv.axon_hooks"] = _m
    import antenv

    antenv.axon_hooks = _m

import numpy as np  # noqa: E402
import bass_rust  # noqa: E402
import concourse.bass as bass  # noqa: E402
import concourse.tile as tile  # noqa: E402
from concourse import mybir  # noqa: E402

F32 = mybir.dt.float32
BF16 = mybir.dt.bfloat16
PSUM = bass.MemorySpace.PSUM
Tanh = mybir.ActivationFunctionType.Tanh
Exp = mybir.ActivationFunctionType.Exp

B, L, RNN, HID, FEAT = 256, 196, 1024, 512, 2048
NCORES = 8
BL = B // NCORES  # batch rows per core (32)
L_HI = 128
L_LO = L - L_HI  # 68
NHC = HID // 128  # 4 h chunks
NRC = RNN // 128  # 8 r chunks
NFQ = FEAT // 512  # 4 psum-bank-sized f chunks
NPAIR = BL // 2  # 16
NQUAD = BL // 4  # 8

AF_BUFS = int(os.environ.get("KERNEL_AF_BUFS", "2"))


def _split_sync(nc):
    """walrus in this image encodes at most ONE semaphore wait and ONE
    semaphore update per instruction; Tile freely emits several. Move the
    extras onto single-wait/single-update NoOp carriers on the same engine
    (engine queues are strict FIFO, so a preceding NoOp's wait gates the
    instruction and a following NoOp's update fires after it completes)."""
    dma_types = {
        "InstDMACopy",
        "InstTensorLoad",
        "InstTensorSave",
        "InstDmaTransposeAnt",
        "InstTensorCopy",
    }
    for f in nc.m.functions:
        for bb in f.blocks:
            new = []
            changed = False
            for ins in bb.instructions:
                si = ins.sync_info
                if si is None:
                    new.append(ins)
                    continue
                waits = list(si.on_wait)
                updates = list(si.on_update)
                if len(waits) <= 1 and len(updates) <= 1:
                    new.append(ins)
                    continue
                changed = True
                tname = type(ins).__name__
                for j, w in enumerate(waits[:-1]):
                    nop = mybir.InstNoOp(name=f"{ins.name}_w{j}", ins=[], outs=[])
                    nop.engine = ins.engine
                    nop.sync_info = bass_rust.SyncInfo(on_wait=[w], on_update=[])
                    new.append(nop)
                keep_w = waits[-1:]
                post_u = []
                keep_u = updates
                if len(updates) > 1:
                    if tname in dma_types:
                        raise RuntimeError(
                            f"DMA instruction {ins.name} carries {len(updates)} "
                            "sem updates; cannot split without changing semantics"
                        )
                    keep_u = updates[:1]
                    post_u = updates[1:]
                ins.sync_info = bass_rust.SyncInfo(on_wait=keep_w, on_update=keep_u)
                new.append(ins)
                for j, u in enumerate(post_u):
                    nop = mybir.InstNoOp(name=f"{ins.name}_u{j}", ins=[], outs=[])
                    nop.engine = ins.engine
                    nop.sync_info = bass_rust.SyncInfo(on_wait=[], on_update=[u])
                    new.append(nop)
            if changed:
                bb.instructions = new


def build_nc(split=True):
    """Inputs arrive host-packed (see _make_in_maps):
      h:       [RNN, BL]                 bf16  (r-major)
      Wah_w:   [RNN, HID]                bf16  (r-major)
      alpha_w: [1, HID]                  f32
      p_att_feats: [128, NPAIR, NHC, 2, L]     bf16 (h%128 on partitions)
      att_hi:  [NQUAD, 128, 4, FEAT]     bf16  (l rows 0..127)
      att_lo:  [NQUAD, L_LO, 4, FEAT]    bf16  (l rows 128..195)
    Output:
      out:     [4, NQUAD, NFQ, 512]      f32   (att[4*qd+j, 512*q+x] =
                                                out[j, qd, q, x])
    """
    nc = bass.Bass()
    h_d = nc.declare_dram_parameter("h", [RNN, BL], BF16, isOutput=False)
    pa_d = nc.declare_dram_parameter(
        "p_att_feats", [128, NPAIR, NHC, 2, L], BF16, isOutput=False
    )
    hi_d = nc.declare_dram_parameter(
        "att_hi", [NQUAD, 128, 4, FEAT], BF16, isOutput=False
    )
    lo_d = nc.declare_dram_parameter(
        "att_lo", [NQUAD, L_LO, 4, FEAT], BF16, isOutput=False
    )
    ww_d = nc.declare_dram_parameter("Wah_w", [RNN, HID], BF16, isOutput=False)
    aw_d = nc.declare_dram_parameter("alpha_w", [1, HID], F32, isOutput=False)
    out_d = nc.declare_dram_parameter("out", [4, NQUAD, NFQ, 512], BF16, isOutput=True)

    with tile.TileContext(nc) as tc:
        with tc.tile_pool(name="singles", bufs=1) as singles:
            wahT = singles.tile([128, NHC, BL], F32)  # WahT[h % 128, hc, b]
            awT = singles.tile([128, NHC], BF16)  # alpha_w^T chunks
            # whole-core p_att stage (bf16, h on partitions): 50KB/partition
            pa_all = singles.tile([128, NPAIR, NHC, 2, L], BF16)
            # exp(scores), 256-wide zero-padded slot per batch so the lo
            # alphaT transpose matmul can span a full 128 output partitions
            expS = singles.tile([1, BL, 256], BF16)
            nc.gpsimd.memset(expS[:], 0.0)
            sums = singles.tile([1, BL], F32)
            rsum = singles.tile([1, BL], F32)
            # 1/sum replicated 32-wide so the alphaT transpose matmuls can
            # produce alpha replicated across 32 columns (-> M=32 phase-2
            # weights that write every partition of their PSUM col group)
            rsum_rep = singles.tile([1, BL, 32], BF16)
            ones_row = singles.tile([1, 32], BF16)
            nc.gpsimd.memset(ones_row[:], 1.0)
            aT_sb = singles.tile([128, BL, 2, 32], BF16)  # alphaT cols (hi, lo)
            # whole-core output stage (bf16): partition 32j holds the att
            # rows of quad-batch j (the other partitions carry copy junk
            # that the output DMAs never read)
            osb_all = singles.tile([128, NQUAD, NFQ, 512], BF16)

            # Batch-loop SBUF pools are allocated FIRST so their zones never
            # overlap the setup pool's -- otherwise the first input DMAs
            # inherit released-zone deps on the setup computation.
            with (
                tc.tile_pool(name="hi", bufs=AF_BUFS) as pool_hi,
                tc.tile_pool(name="lo", bufs=AF_BUFS) as pool_lo,
                tc.tile_pool(name="ea", bufs=3) as pool_ea,
                tc.tile_pool(name="e", bufs=3) as pool_e,
            ):
                # ---------------- setup: weights ----------------
                with (
                    tc.tile_pool(name="setup_sb", bufs=1) as ssb,
                    tc.tile_pool(name="setup_ps", bufs=2, space=PSUM) as sps,
                    tc.tile_pool(name="setup_acc", bufs=1, space=PSUM) as sacc,
                ):
                    hT = ssb.tile([128, NRC, BL], BF16)
                    nc.sync.dma_start(
                        hT[:], h_d[:].rearrange("(rc p) b -> p rc b", p=128)
                    )
                    wwT = ssb.tile([128, NRC, HID], BF16)
                    nc.sync.dma_start(
                        wwT[:], ww_d[:].rearrange("(rc p) c -> p rc c", p=128)
                    )
                    aw_sb = ssb.tile([1, HID], F32)
                    nc.sync.dma_start(aw_sb[:], aw_d[:])
                    ones11 = ssb.tile([1, 1], F32)
                    nc.gpsimd.memset(ones11[:], 1.0)

                    # alpha_w^T columns (bf16 to match bf16 e tiles)
                    for hc in range(NHC):
                        ps = sps.tile([128, 1], F32, tag="aw")
                        nc.tensor.matmul(
                            ps[:],
                            aw_sb[0:1, hc * 128 : (hc + 1) * 128],
                            ones11[:],
                            start=True,
                            stop=True,
                        )
                        nc.vector.tensor_copy(awT[:, hc : hc + 1], ps[:])

                    # WahT[h, b] = sum_r Wah_w[h, r] * h[b, r]
                    wahT_ps = [
                        sacc.tile([128, BL], F32, tag=f"acc{hc}", name=f"wahT_ps{hc}")
                        for hc in range(NHC)
                    ]
                    for rc in range(NRC):
                        for hc in range(NHC):
                            nc.tensor.matmul(
                                wahT_ps[hc][:],
                                wwT[:, rc, hc * 128 : (hc + 1) * 128],
                                hT[:, rc, :],
                                start=(rc == 0),
                                stop=(rc == NRC - 1),
                            )
                    for hc in range(NHC):
                        nc.vector.tensor_copy(wahT[:, hc, :], wahT_ps[hc][:])

                # p_att stream: 4 chunk DMAs into the whole-core stage, on
                # the SWDGE (gpsimd) queue so they don't contend with the
                # att_feats stream on the HWDGE rings
                for c in range(4):
                    nc.gpsimd.dma_start(
                        pa_all[:, 4 * c : 4 * c + 4], pa_d[:, 4 * c : 4 * c + 4]
                    )

                # ---------------- streaming batch loop ----------------
                with (
                    tc.tile_pool(name="sc_ps", bufs=2, space=PSUM) as pool_sc,
                    tc.tile_pool(name="aT_ps", bufs=2, space=PSUM) as pool_aT,
                    tc.tile_pool(name="ao_ps", bufs=1, space=PSUM) as pool_ao,
                ):
                    def phase2(quad, af_hi, af_lo):
                        for q in range(NFQ):
                            ao = pool_ao.tile([128, 512], F32, tag=f"q{q}")
                            fsl = slice(q * 512, (q + 1) * 512)
                            for j in range(4):
                                nc.tensor.matmul(
                                    ao[32 * j : 32 * j + 32, :],
                                    aT_sb[:, 4 * quad + j, 0],
                                    af_hi[:, j, fsl],
                                    start=True,
                                    stop=False,
                                    tile_position=(0, 32 * j),
                                )
                                nc.tensor.matmul(
                                    ao[32 * j : 32 * j + 32, :],
                                    aT_sb[0:L_LO, 4 * quad + j, 1],
                                    af_lo[0:L_LO, j, fsl],
                                    start=False,
                                    stop=True,
                                    tile_position=(0, 32 * j),
                                )
                            # full-width copy: partition-strided APs are
                            # illegal on compute engines; copying all 128
                            # lanes costs the same (per-lane elements).
                            # Split across DVE and ACT to balance load.
                            if q % 2 == 0:
                                nc.vector.tensor_copy(
                                    osb_all[:, quad, q, :], ao[:]
                                )
                            else:
                                nc.scalar.copy(osb_all[:, quad, q, :], ao[:])

                    # Software-pipelined: phase 2 of quad q-1 is emitted
                    # AFTER phase 1 of quad q, so the PE queue never
                    # head-of-line blocks on the af DMA of the current quad
                    # (the DMA issued a full quad-period before phase2
                    # consumes it).
                    prev = None
                    for quad in range(NQUAD):
                        # hi on the SP HWDGE ring, lo on the GpSimd SWDGE
                        # ring: two queues keep more SDMA descriptors in
                        # flight (one queue measured only 143 GB/s on this
                        # stream). GpSimd runs no compute in this kernel, so
                        # a WAR-waiting DMA at its queue head blocks nothing
                        # (on Scalar it stalled the tanh/exp stream).
                        af_hi = pool_hi.tile([128, 4, FEAT], BF16, tag="hi")
                        nc.sync.dma_start(af_hi[:], hi_d[quad])
                        af_lo = pool_lo.tile([L_LO, 4, FEAT], BF16, tag="lo")
                        nc.gpsimd.dma_start(af_lo[:], lo_d[quad])

                        # ---- phase 1 for the quad's two pairs ----
                        for pp in range(2):
                            p = 2 * quad + pp
                            b0 = 2 * p
                            # Wah broadcast-adds on DVE (265ns each true
                            # cost; GpSimd's generic path is 3.6us -- keep
                            # it off), then ONE tanh over the whole pair
                            # tile so ACT pays the 352-cycle instruction
                            # overhead once instead of 8 times
                            ea = pool_ea.tile([128, NHC, 2, L], BF16)
                            for hc in range(NHC):
                                for jb in range(2):
                                    b = b0 + jb
                                    nc.vector.tensor_scalar_add(
                                        ea[:, hc, jb, :],
                                        pa_all[:, p, hc, jb, :],
                                        wahT[:, hc, b : b + 1],
                                    )
                            e = pool_e.tile([128, NHC, 2, L], BF16)
                            nc.scalar.activation(e[:], ea[:], Tanh)

                            sc = pool_sc.tile([1, 2, L], F32)
                            for hc in range(NHC):
                                nc.tensor.matmul(
                                    sc[:],
                                    awT[:, hc : hc + 1],
                                    e[:, hc],
                                    start=(hc == 0),
                                    stop=(hc == NHC - 1),
                                )

                            for jb in range(2):
                                b = b0 + jb
                                nc.scalar.activation(
                                    expS[0:1, b, 0:L],
                                    sc[0:1, jb, :],
                                    Exp,
                                    accum_out=sums[0:1, b : b + 1],
                                )
                                nc.vector.reciprocal(
                                    rsum[0:1, b : b + 1], sums[0:1, b : b + 1]
                                )
                                nc.vector.tensor_scalar_mul(
                                    rsum_rep[0:1, b, :],
                                    ones_row[:],
                                    rsum[0:1, b : b + 1],
                                )
                                # alphaT columns via K=1 matmuls; rhs = the
                                # replicated 1/sum row, folding the softmax
                                # normalization in and replicating alpha to
                                # 32 columns
                                aT = pool_aT.tile([128, 2, 32], F32)
                                nc.tensor.matmul(
                                    aT[:, 0, :],
                                    expS[0:1, b, 0:128],
                                    rsum_rep[0:1, b, :],
                                    start=True,
                                    stop=True,
                                )
                                nc.tensor.matmul(
                                    aT[:, 1, :],
                                    expS[0:1, b, 128:256],
                                    rsum_rep[0:1, b, :],
                                    start=True,
                                    stop=True,
                                )
                                nc.vector.tensor_copy(aT_sb[:, b], aT[:])

                        # ---- phase 2 for the PREVIOUS quad ----
                        if prev is not None:
                            phase2(*prev)
                        prev = (quad, af_hi, af_lo)
                    phase2(*prev)

                    # final output DMAs: partition 32j carries quad-batch j
                    for j in range(4):
                        nc.sync.dma_start(
                            out_d[j : j + 1], osb_all[32 * j : 32 * j + 1]
                        )

    if split:
        _split_sync(nc)
    return nc


_NC_CACHE = None


def _get_nc():
    global _NC_CACHE
    if _NC_CACHE is None:
        _NC_CACHE = build_nc()
    return _NC_CACHE


def _make_in_maps(h, att_feats, p_att_feats, Wah_w, alpha_w):
    import ml_dtypes

    bf = ml_dtypes.bfloat16
    h = np.ascontiguousarray(h, dtype=np.float32)
    att_feats = np.ascontiguousarray(att_feats, dtype=np.float32)
    p_att_feats = np.ascontiguousarray(p_att_feats, dtype=np.float32)
    wwT_host = np.ascontiguousarray(Wah_w.T).astype(bf)  # [RNN, HID]
    alpha_w = np.ascontiguousarray(alpha_w, dtype=np.float32)
    in_maps = []
    for i in range(NCORES):
        sl = slice(i * BL, (i + 1) * BL)
        # p_att: [BL, L, HID] -> [128, NPAIR, NHC, 2, L]
        pa = (
            p_att_feats[sl]
            .reshape(NPAIR, 2, L, NHC, 128)
            .transpose(4, 0, 3, 1, 2)
            .astype(bf)
        )
        af = att_feats[sl].reshape(NQUAD, 4, L, FEAT)
        af_hi = np.ascontiguousarray(af[:, :, :L_HI].transpose(0, 2, 1, 3)).astype(bf)
        af_lo = np.ascontiguousarray(af[:, :, L_HI:].transpose(0, 2, 1, 3)).astype(bf)
        in_maps.append(
            {
                "h": np.ascontiguousarray(h[sl].T).astype(bf),
                "p_att_feats": np.ascontiguousarray(pa),
                "att_hi": af_hi,
                "att_lo": af_lo,
                "Wah_w": wwT_host,
                "alpha_w": alpha_w,
            }
        )
    return in_maps


def _unpack_out(o):
    """[4, NQUAD, NFQ, 512] -> [BL, FEAT]"""
    return np.ascontiguousarray(
        np.asarray(o, dtype=np.float32).transpose(1, 0, 2, 3).reshape(BL, FEAT)
    )


def run_spmd(h, att_feats, p_att_feats, Wah_w, alpha_w, trace=False):
    """Run the SPMD kernel; returns (full_output, BassKernelResults)."""
    from concourse.bass_utils import run_bass_kernel_spmd

    nc = _get_nc()
    in_maps = _make_in_maps(h, att_feats, p_att_feats, Wah_w, alpha_w)
    res = run_bass_kernel_spmd(nc, in_maps, list(range(NCORES)), trace=trace)
    out = np.concatenate(
        [_unpack_out(res.results[i]["out"]) for i in range(NCORES)], axis=0
    )
    return out, res


def kernel(h, att_feats, p_att_feats, Wah_w, alpha_w):
    out, _ = run_spmd(h, att_feats, p_att_feats, Wah_w, alpha_w, trace=False)
    return out


# revision 39
# speedup vs baseline: 1.0139x; 1.0059x over previous
"""Trainium2 Bass kernel for additive-attention pooling (v2).

Computation (per batch row b):
    Wah   = h @ Wah_w.T                         [B, HID]
    e     = tanh(Wah[:, None, :] + p_att_feats) [B, L, HID]
    s     = e @ alpha_w[0]                      [B, L]
    alpha = softmax(s, -1)                      [B, L]
    att   = sum_l alpha[b, l] * att_feats[b, l, :]   [B, FEAT]

Sharding: pure data parallel over the batch dim, 32 rows per core on 8
NeuronCores; the small Wah_w / alpha_w weights are replicated.

v2 dataflow changes vs the first working kernel (264 us):
  * p_att_feats arrives host-TRANSPOSED ([h%128 partitions, pair, hc,
    jb, l] bf16) so the kernel does ZERO PE transposes (the old kernel
    spent ~2/3 of its TensorE time on transpose matmuls and ran the PE
    at half clock from HAM oscillation).
  * The Wah broadcast-add moves off the ACT bias path onto DVE+GpSimd
    tensor_scalar adds (8 small adds per pair, engines alternated),
    followed by ONE tanh activation over the whole [128, 4*2*196] pair
    tile -- ACT instruction count drops 128 -> 16 for the tanh work.
  * Phase 2 (att = alpha^T @ att_feats) runs as 4-way column-tiled
    matmuls: batch j of a quad occupies PE column-group j
    (tile_position=(0,32j), alpha column as the stationary operand), so
    4 batches stream their att_feats concurrently and the PSUM output
    lands on partitions {0,32,64,96} -- the PSUM->SBUF copies are
    [4, 512] (4 active lanes) instead of [1, 512] (1 lane), which
    removes the ~80 us of single-lane copies the old kernel paid.
  * Outputs stage per quad and DMA out via strided-partition APs.

All data stays bf16 on the wire (fp8 was measured: rel_norm 2.7e-2 on
att_feats -- too close to the 2e-2 gate).

The walrus build in this image accepts only one semaphore wait and one
update per instruction; _split_sync() post-processes the scheduled BIR
to spread Tile's multi-wait/multi-update sync info onto NoOp carriers.
"""

import os
import sys
import types

sys.path.insert(0, "/opt/trn_rl_repo")

# This image's antenv package lacks axon_hooks; provide it so
# concourse.bass_utils can import it (trace path) without crashing.
if "antenv.axon_hooks" not in sys.modules:
    _m = types.ModuleType("antenv.axon_hooks")

    def _set_hook(h):
        _m._hook = h

    def _get_hook():
        return getattr(_m, "_hook", None)

    _m.set_axon_ntff_profile_hook = _set_hook
    _m.get_axon_ntff_profile_hook = _get_hook
    sys.modules["antenv.axon_hooks"] = _m
    import antenv

    antenv.axon_hooks = _m

import numpy as np  # noqa: E402
import bass_rust  # noqa: E402
import concourse.bass as bass  # noqa: E402
import concourse.tile as tile  # noqa: E402
from concourse import mybir  # noqa: E402

F32 = mybir.dt.float32
BF16 = mybir.dt.bfloat16
PSUM = bass.MemorySpace.PSUM
Tanh = mybir.ActivationFunctionType.Tanh
Exp = mybir.ActivationFunctionType.Exp

B, L, RNN, HID, FEAT = 256, 196, 1024, 512, 2048
NCORES = 8
BL = B // NCORES  # batch rows per core (32)
L_HI = 128
L_LO = L - L_HI  # 68
NHC = HID // 128  # 4 h chunks
NRC = RNN // 128  # 8 r chunks
NFQ = FEAT // 512  # 4 psum-bank-sized f chunks
NPAIR = BL // 2  # 16
NQUAD = BL // 4  # 8
NOCT = BL // 8  # 4

AF_BUFS = int(os.environ.get("KERNEL_AF_BUFS", "2"))


def _split_sync(nc):
    """walrus in this image encodes at most ONE semaphore wait and ONE
    semaphore update per instruction; Tile freely emits several. Move the
    extras onto single-wait/single-update NoOp carriers on the same engine
    (engine queues are strict FIFO, so a preceding NoOp's wait gates the
    instruction and a following NoOp's update fires after it completes)."""
    dma_types = {
        "InstDMACopy",
        "InstTensorLoad",
        "InstTensorSave",
        "InstDmaTransposeAnt",
        "InstTensorCopy",
    }
    for f in nc.m.functions:
        for bb in f.blocks:
            new = []
            changed = False
            for ins in bb.instructions:
                si = ins.sync_info
                if si is None:
                    new.append(ins)
                    continue
                waits = list(si.on_wait)
                updates = list(si.on_update)
                if len(waits) <= 1 and len(updates) <= 1:
                    new.append(ins)
                    continue
                changed = True
                tname = type(ins).__name__
                for j, w in enumerate(waits[:-1]):
                    nop = mybir.InstNoOp(name=f"{ins.name}_w{j}", ins=[], outs=[])
                    nop.engine = ins.engine
                    nop.sync_info = bass_rust.SyncInfo(on_wait=[w], on_update=[])
                    new.append(nop)
                keep_w = waits[-1:]
                post_u = []
                keep_u = updates
                if len(updates) > 1:
                    if tname in dma_types:
                        raise RuntimeError(
                            f"DMA instruction {ins.name} carries {len(updates)} "
                            "sem updates; cannot split without changing semantics"
                        )
                    keep_u = updates[:1]
                    post_u = updates[1:]
                ins.sync_info = bass_rust.SyncInfo(on_wait=keep_w, on_update=keep_u)
                new.append(ins)
                for j, u in enumerate(post_u):
                    nop = mybir.InstNoOp(name=f"{ins.name}_u{j}", ins=[], outs=[])
                    nop.engine = ins.engine
                    nop.sync_info = bass_rust.SyncInfo(on_wait=[], on_update=[u])
                    new.append(nop)
            if changed:
                bb.instructions = new


def build_nc(split=True):
    """Inputs arrive host-packed (see _make_in_maps):
      h:       [RNN, BL]                 bf16  (r-major)
      Wah_w:   [RNN, HID]                bf16  (r-major)
      alpha_w: [1, HID]                  f32
      p_att_feats: [128, NPAIR, NHC, 2, L]     bf16 (h%128 on partitions)
      att_hi:  [NQUAD, 128, 4, FEAT]     bf16  (l rows 0..127)
      att_lo:  [NQUAD, L_LO, 4, FEAT]    bf16  (l rows 128..195)
    Output:
      out:     [4, NQUAD, NFQ, 512]      f32   (att[4*qd+j, 512*q+x] =
                                                out[j, qd, q, x])
    """
    nc = bass.Bass()
    h_d = nc.declare_dram_parameter("h", [RNN, BL], BF16, isOutput=False)
    pa_d = nc.declare_dram_parameter(
        "p_att_feats", [NOCT, 128, 4, NHC, 2, L], BF16, isOutput=False
    )
    hi_d = nc.declare_dram_parameter(
        "att_hi", [NOCT, 128, 8, FEAT], BF16, isOutput=False
    )
    lo_d = nc.declare_dram_parameter(
        "att_lo", [NOCT, L_LO, 8, FEAT], BF16, isOutput=False
    )
    ww_d = nc.declare_dram_parameter("Wah_w", [RNN, HID], BF16, isOutput=False)
    aw_d = nc.declare_dram_parameter("alpha_w", [1, HID], F32, isOutput=False)
    out_d = nc.declare_dram_parameter("out", [4, NQUAD, NFQ, 512], BF16, isOutput=True)

    with tile.TileContext(nc) as tc:
        with tc.tile_pool(name="singles", bufs=1) as singles:
            wahT = singles.tile([128, NHC, BL], F32)  # WahT[h % 128, hc, b]
            awT = singles.tile([128, NHC], BF16)  # alpha_w^T chunks
            # exp(scores), 256-wide zero-padded slot per batch so the lo
            # alphaT transpose matmul can span a full 128 output partitions
            expS = singles.tile([1, BL, 256], BF16)
            nc.gpsimd.memset(expS[:], 0.0)
            sums = singles.tile([1, BL], F32)
            rsum = singles.tile([1, BL], F32)
            # 1/sum replicated 32-wide so the alphaT transpose matmuls can
            # produce alpha replicated across 32 columns (-> M=32 phase-2
            # weights that write every partition of their PSUM col group)
            rsum_rep = singles.tile([1, BL, 32], BF16)
            ones_row = singles.tile([1, 32], BF16)
            nc.gpsimd.memset(ones_row[:], 1.0)
            aT_sb = singles.tile([128, BL, 2, 32], BF16)  # alphaT cols (hi, lo)

            # Batch-loop SBUF pools are allocated FIRST so their zones never
            # overlap the setup pool's -- otherwise the first input DMAs
            # inherit released-zone deps on the setup computation.
            with (
                tc.tile_pool(name="hi", bufs=AF_BUFS) as pool_hi,
                tc.tile_pool(name="lo", bufs=AF_BUFS) as pool_lo,
                tc.tile_pool(name="pa", bufs=2) as pool_pa,
                tc.tile_pool(name="ea", bufs=2) as pool_ea,
                tc.tile_pool(name="e", bufs=2) as pool_e,
                tc.tile_pool(name="osb", bufs=2) as pool_osb,
            ):
                # ---------------- setup: weights ----------------
                with (
                    tc.tile_pool(name="setup_sb", bufs=1) as ssb,
                    tc.tile_pool(name="setup_ww", bufs=2) as sww,
                    tc.tile_pool(name="setup_ps", bufs=2, space=PSUM) as sps,
                    tc.tile_pool(name="setup_acc", bufs=1, space=PSUM) as sacc,
                ):
                    hT = ssb.tile([128, NRC, BL], BF16)
                    nc.sync.dma_start(
                        hT[:], h_d[:].rearrange("(rc p) b -> p rc b", p=128)
                    )
                    aw_sb = ssb.tile([1, HID], BF16)
                    # f32 -> bf16 cast during DMA is SWDGE-only
                    nc.gpsimd.dma_start(aw_sb[:], aw_d[:])
                    ones11 = ssb.tile([1, 1], BF16)
                    nc.gpsimd.memset(ones11[:], 1.0)

                    # alpha_w^T columns (bf16 to match bf16 e tiles)
                    for hc in range(NHC):
                        ps = sps.tile([128, 1], F32, tag="aw")
                        nc.tensor.matmul(
                            ps[:],
                            aw_sb[0:1, hc * 128 : (hc + 1) * 128],
                            ones11[:],
                            start=True,
                            stop=True,
                        )
                        nc.vector.tensor_copy(awT[:, hc : hc + 1], ps[:])

                    # WahT[h, b] = sum_r Wah_w[h, r] * h[b, r]; the Wah_w
                    # chunks stream through a small 2-buf pool
                    wahT_ps = [
                        sacc.tile([128, BL], F32, tag=f"acc{hc}", name=f"wahT_ps{hc}")
                        for hc in range(NHC)
                    ]
                    ww_r = ww_d[:].rearrange("(rc p) c -> p rc c", p=128)
                    for rc in range(NRC):
                        wwc = sww.tile([128, HID], BF16, tag="ww")
                        nc.sync.dma_start(wwc[:], ww_r[:, rc])
                        for hc in range(NHC):
                            nc.tensor.matmul(
                                wahT_ps[hc][:],
                                wwc[:, hc * 128 : (hc + 1) * 128],
                                hT[:, rc, :],
                                start=(rc == 0),
                                stop=(rc == NRC - 1),
                            )
                    for hc in range(NHC):
                        nc.vector.tensor_copy(wahT[:, hc, :], wahT_ps[hc][:])

                # ---------------- streaming batch loop ----------------
                with (
                    tc.tile_pool(name="sc_ps", bufs=2, space=PSUM) as pool_sc,
                    tc.tile_pool(name="aT_ps", bufs=2, space=PSUM) as pool_aT,
                    tc.tile_pool(name="ao_ps", bufs=1, space=PSUM) as pool_ao,
                ):
                    def phase2(oct_, af_hi, af_lo, osb):
                        for r in range(2):
                            quad = 2 * oct_ + r
                            for q in range(NFQ):
                                ao = pool_ao.tile([128, 512], F32, tag=f"q{q}")
                                fsl = slice(q * 512, (q + 1) * 512)
                                for j in range(4):
                                    nc.tensor.matmul(
                                        ao[32 * j : 32 * j + 32, :],
                                        aT_sb[:, 4 * quad + j, 0],
                                        af_hi[:, 4 * r + j, fsl],
                                        start=True,
                                        stop=False,
                                        tile_position=(0, 32 * j),
                                    )
                                    nc.tensor.matmul(
                                        ao[32 * j : 32 * j + 32, :],
                                        aT_sb[0:L_LO, 4 * quad + j, 1],
                                        af_lo[0:L_LO, 4 * r + j, fsl],
                                        start=False,
                                        stop=True,
                                        tile_position=(0, 32 * j),
                                    )
                                # full-width copy: partition-strided APs are
                                # illegal on compute engines; copying all
                                # 128 lanes costs the same (per-lane elems).
                                # Split across DVE and ACT to balance load.
                                if q % 2 == 0:
                                    nc.vector.tensor_copy(osb[:, r, q, :], ao[:])
                                else:
                                    nc.scalar.copy(osb[:, r, q, :], ao[:])
                        # output DMAs: partition 32j carries quad-batch j
                        for j in range(4):
                            nc.sync.dma_start(
                                out_d[j : j + 1, 2 * oct_ : 2 * oct_ + 2],
                                osb[32 * j : 32 * j + 1],
                            )

                    # Software-pipelined: phase 2 of oct o-1 is emitted
                    # AFTER phase 1 of oct o, so the PE queue never
                    # head-of-line blocks on the af DMA of the current oct
                    # (that DMA was issued a full oct-period before phase2
                    # consumes it). Oct-sized af transfers give 32KB
                    # descriptors (16KB ones measured only ~15 GB/s per
                    # SDMA engine).
                    prev = None
                    for oct_ in range(NOCT):
                        af_hi = pool_hi.tile([128, 8, FEAT], BF16, tag="hi")
                        nc.sync.dma_start(af_hi[:], hi_d[oct_])
                        af_lo = pool_lo.tile([L_LO, 8, FEAT], BF16, tag="lo")
                        nc.sync.dma_start(af_lo[:], lo_d[oct_])
                        pa_oct = pool_pa.tile([128, 4, NHC, 2, L], BF16, tag="pa")
                        nc.gpsimd.dma_start(pa_oct[:], pa_d[oct_])
                        osb = pool_osb.tile([128, 2, NFQ, 512], BF16, tag="osb")

                        # ---- phase 1 for the oct's four pairs ----
                        for pp in range(4):
                            p = 4 * oct_ + pp
                            b0 = 2 * p
                            # Wah broadcast-adds on DVE (265ns each true
                            # cost; GpSimd's generic path is 3.6us -- keep
                            # it off), then ONE tanh over the whole pair
                            # tile so ACT pays the 352-cycle instruction
                            # overhead once instead of 8 times
                            ea = pool_ea.tile([128, NHC, 2, L], BF16)
                            for hc in range(NHC):
                                for jb in range(2):
                                    b = b0 + jb
                                    nc.vector.tensor_scalar_add(
                                        ea[:, hc, jb, :],
                                        pa_oct[:, pp, hc, jb, :],
                                        wahT[:, hc, b : b + 1],
                                    )
                            e = pool_e.tile([128, NHC, 2, L], BF16)
                            nc.scalar.activation(e[:], ea[:], Tanh)

                            sc = pool_sc.tile([1, 2, L], F32)
                            for hc in range(NHC):
                                nc.tensor.matmul(
                                    sc[:],
                                    awT[:, hc : hc + 1],
                                    e[:, hc],
                                    start=(hc == 0),
                                    stop=(hc == NHC - 1),
                                )

                            for jb in range(2):
                                b = b0 + jb
                                nc.scalar.activation(
                                    expS[0:1, b, 0:L],
                                    sc[0:1, jb, :],
                                    Exp,
                                    accum_out=sums[0:1, b : b + 1],
                                )
                                nc.vector.reciprocal(
                                    rsum[0:1, b : b + 1], sums[0:1, b : b + 1]
                                )
                                nc.vector.tensor_scalar_mul(
                                    rsum_rep[0:1, b, :],
                                    ones_row[:],
                                    rsum[0:1, b : b + 1],
                                )
                                # alphaT columns via K=1 matmuls; rhs = the
                                # replicated 1/sum row, folding the softmax
                                # normalization in and replicating alpha to
                                # 32 columns
                                aT = pool_aT.tile([128, 2, 32], F32)
                                nc.tensor.matmul(
                                    aT[:, 0, :],
                                    expS[0:1, b, 0:128],
                                    rsum_rep[0:1, b, :],
                                    start=True,
                                    stop=True,
                                )
                                nc.tensor.matmul(
                                    aT[:, 1, :],
                                    expS[0:1, b, 128:256],
                                    rsum_rep[0:1, b, :],
                                    start=True,
                                    stop=True,
                                )
                                nc.vector.tensor_copy(aT_sb[:, b], aT[:])

                        # ---- phase 2 for the PREVIOUS quad ----
                        if prev is not None:
                            phase2(*prev)
                        prev = (oct_, af_hi, af_lo, osb)
                    phase2(*prev)

    if split:
        _split_sync(nc)
    return nc


_NC_CACHE = None


def _get_nc():
    global _NC_CACHE
    if _NC_CACHE is None:
        _NC_CACHE = build_nc()
    return _NC_CACHE


def _make_in_maps(h, att_feats, p_att_feats, Wah_w, alpha_w):
    import ml_dtypes

    bf = ml_dtypes.bfloat16
    h = np.ascontiguousarray(h, dtype=np.float32)
    att_feats = np.ascontiguousarray(att_feats, dtype=np.float32)
    p_att_feats = np.ascontiguousarray(p_att_feats, dtype=np.float32)
    wwT_host = np.ascontiguousarray(Wah_w.T).astype(bf)  # [RNN, HID]
    alpha_w = np.ascontiguousarray(alpha_w, dtype=np.float32)
    in_maps = []
    for i in range(NCORES):
        sl = slice(i * BL, (i + 1) * BL)
        # p_att: [BL, L, HID] -> [NOCT, 128, 4pairs, NHC, 2, L]
        pa = (
            p_att_feats[sl]
            .reshape(NOCT, 4, 2, L, NHC, 128)
            .transpose(0, 5, 1, 4, 2, 3)
            .astype(bf)
        )
        af = att_feats[sl].reshape(NOCT, 8, L, FEAT)
        af_hi = np.ascontiguousarray(af[:, :, :L_HI].transpose(0, 2, 1, 3)).astype(bf)
        af_lo = np.ascontiguousarray(af[:, :, L_HI:].transpose(0, 2, 1, 3)).astype(bf)
        in_maps.append(
            {
                "h": np.ascontiguousarray(h[sl].T).astype(bf),
                "p_att_feats": np.ascontiguousarray(pa),
                "att_hi": af_hi,
                "att_lo": af_lo,
                "Wah_w": wwT_host,
                "alpha_w": alpha_w,
            }
        )
    return in_maps


def _unpack_out(o):
    """[4, NQUAD, NFQ, 512] -> [BL, FEAT]"""
    return np.ascontiguousarray(
        np.asarray(o, dtype=np.float32).transpose(1, 0, 2, 3).reshape(BL, FEAT)
    )


def run_spmd(h, att_feats, p_att_feats, Wah_w, alpha_w, trace=False):
    """Run the SPMD kernel; returns (full_output, BassKernelResults)."""
    from concourse.bass_utils import run_bass_kernel_spmd

    nc = _get_nc()
    in_maps = _make_in_maps(h, att_feats, p_att_feats, Wah_w, alpha_w)
    res = run_bass_kernel_spmd(nc, in_maps, list(range(NCORES)), trace=trace)
    out = np.concatenate(
        [_unpack_out(res.results[i]["out"]) for i in range(NCORES)], axis=0
    )
    return out, res


def kernel(h, att_feats, p_att_feats, Wah_w, alpha_w):
    out, _ = run_spmd(h, att_feats, p_att_feats, Wah_w, alpha_w, trace=False)
    return out


# revision 41
# speedup vs baseline: 1.2483x; 1.2313x over previous
"""Trainium2 Bass kernel for additive-attention pooling (v2).

Computation (per batch row b):
    Wah   = h @ Wah_w.T                         [B, HID]
    e     = tanh(Wah[:, None, :] + p_att_feats) [B, L, HID]
    s     = e @ alpha_w[0]                      [B, L]
    alpha = softmax(s, -1)                      [B, L]
    att   = sum_l alpha[b, l] * att_feats[b, l, :]   [B, FEAT]

Sharding: pure data parallel over the batch dim, 32 rows per core on 8
NeuronCores; the small Wah_w / alpha_w weights are replicated.

v2 dataflow changes vs the first working kernel (264 us):
  * p_att_feats arrives host-TRANSPOSED ([h%128 partitions, pair, hc,
    jb, l] bf16) so the kernel does ZERO PE transposes (the old kernel
    spent ~2/3 of its TensorE time on transpose matmuls and ran the PE
    at half clock from HAM oscillation).
  * The Wah broadcast-add moves off the ACT bias path onto DVE+GpSimd
    tensor_scalar adds (8 small adds per pair, engines alternated),
    followed by ONE tanh activation over the whole [128, 4*2*196] pair
    tile -- ACT instruction count drops 128 -> 16 for the tanh work.
  * Phase 2 (att = alpha^T @ att_feats) runs as 4-way column-tiled
    matmuls: batch j of a quad occupies PE column-group j
    (tile_position=(0,32j), alpha column as the stationary operand), so
    4 batches stream their att_feats concurrently and the PSUM output
    lands on partitions {0,32,64,96} -- the PSUM->SBUF copies are
    [4, 512] (4 active lanes) instead of [1, 512] (1 lane), which
    removes the ~80 us of single-lane copies the old kernel paid.
  * Outputs stage per quad and DMA out via strided-partition APs.

All data stays bf16 on the wire (fp8 was measured: rel_norm 2.7e-2 on
att_feats -- too close to the 2e-2 gate).

The walrus build in this image accepts only one semaphore wait and one
update per instruction; _split_sync() post-processes the scheduled BIR
to spread Tile's multi-wait/multi-update sync info onto NoOp carriers.
"""

import os
import sys
import types

sys.path.insert(0, "/opt/trn_rl_repo")

# This image's antenv package lacks axon_hooks; provide it so
# concourse.bass_utils can import it (trace path) without crashing.
if "antenv.axon_hooks" not in sys.modules:
    _m = types.ModuleType("antenv.axon_hooks")

    def _set_hook(h):
        _m._hook = h

    def _get_hook():
        return getattr(_m, "_hook", None)

    _m.set_axon_ntff_profile_hook = _set_hook
    _m.get_axon_ntff_profile_hook = _get_hook
    sys.modules["antenv.axon_hooks"] = _m
    import antenv

    antenv.axon_hooks = _m

import numpy as np  # noqa: E402
import bass_rust  # noqa: E402
import concourse.bass as bass  # noqa: E402
import concourse.tile as tile  # noqa: E402
from concourse import mybir  # noqa: E402

F32 = mybir.dt.float32
BF16 = mybir.dt.bfloat16
PSUM = bass.MemorySpace.PSUM
Tanh = mybir.ActivationFunctionType.Tanh
Exp = mybir.ActivationFunctionType.Exp

B, L, RNN, HID, FEAT = 256, 196, 1024, 512, 2048
NCORES = 8
BL = B // NCORES  # batch rows per core (32)
L_HI = 128
L_LO = L - L_HI  # 68
NHC = HID // 128  # 4 h chunks
NRC = RNN // 128  # 8 r chunks
NFQ = FEAT // 512  # 4 psum-bank-sized f chunks
NPAIR = BL // 2  # 16
NQUAD = BL // 4  # 8
NOCT = BL // 8  # 4
# lo rows padded 68 -> 80 (multiple of 16): the DMA fans across
# largest-divisor-of-desc-count<=16 engines, so 68 descriptors land on
# only 4 SDMA engines while 80 spread over 16
LPAD_LO = 80

AF_BUFS = int(os.environ.get("KERNEL_AF_BUFS", "2"))


def _split_sync(nc):
    """walrus in this image encodes at most ONE semaphore wait and ONE
    semaphore update per instruction; Tile freely emits several. Move the
    extras onto single-wait/single-update NoOp carriers on the same engine
    (engine queues are strict FIFO, so a preceding NoOp's wait gates the
    instruction and a following NoOp's update fires after it completes)."""
    dma_types = {
        "InstDMACopy",
        "InstTensorLoad",
        "InstTensorSave",
        "InstDmaTransposeAnt",
        "InstTensorCopy",
    }
    for f in nc.m.functions:
        for bb in f.blocks:
            new = []
            changed = False
            for ins in bb.instructions:
                si = ins.sync_info
                if si is None:
                    new.append(ins)
                    continue
                waits = list(si.on_wait)
                updates = list(si.on_update)
                if len(waits) <= 1 and len(updates) <= 1:
                    new.append(ins)
                    continue
                changed = True
                tname = type(ins).__name__
                for j, w in enumerate(waits[:-1]):
                    nop = mybir.InstNoOp(name=f"{ins.name}_w{j}", ins=[], outs=[])
                    nop.engine = ins.engine
                    nop.sync_info = bass_rust.SyncInfo(on_wait=[w], on_update=[])
                    new.append(nop)
                keep_w = waits[-1:]
                post_u = []
                keep_u = updates
                if len(updates) > 1:
                    if tname in dma_types:
                        raise RuntimeError(
                            f"DMA instruction {ins.name} carries {len(updates)} "
                            "sem updates; cannot split without changing semantics"
                        )
                    keep_u = updates[:1]
                    post_u = updates[1:]
                ins.sync_info = bass_rust.SyncInfo(on_wait=keep_w, on_update=keep_u)
                new.append(ins)
                for j, u in enumerate(post_u):
                    nop = mybir.InstNoOp(name=f"{ins.name}_u{j}", ins=[], outs=[])
                    nop.engine = ins.engine
                    nop.sync_info = bass_rust.SyncInfo(on_wait=[], on_update=[u])
                    new.append(nop)
            if changed:
                bb.instructions = new


def build_nc(split=True):
    """Inputs arrive host-packed (see _make_in_maps):
      h:       [RNN, BL]                 bf16  (r-major)
      Wah_w:   [RNN, HID]                bf16  (r-major)
      alpha_w: [1, HID]                  f32
      p_att_feats: [128, NPAIR, NHC, 2, L]     bf16 (h%128 on partitions)
      att_hi:  [NQUAD, 128, 4, FEAT]     bf16  (l rows 0..127)
      att_lo:  [NQUAD, L_LO, 4, FEAT]    bf16  (l rows 128..195)
    Output:
      out:     [4, NQUAD, NFQ, 512]      f32   (att[4*qd+j, 512*q+x] =
                                                out[j, qd, q, x])
    """
    nc = bass.Bass()
    h_d = nc.declare_dram_parameter("h", [RNN, BL], BF16, isOutput=False)
    pa_d = nc.declare_dram_parameter(
        "p_att_feats", [NOCT, 128, 4, NHC, 2, L], BF16, isOutput=False
    )
    hi_d = nc.declare_dram_parameter(
        "att_hi", [NOCT, 128, 8, FEAT], BF16, isOutput=False
    )
    lo_d = nc.declare_dram_parameter(
        "att_lo", [NOCT, LPAD_LO, 8, FEAT], BF16, isOutput=False
    )
    ww_d = nc.declare_dram_parameter("Wah_w", [RNN, HID], BF16, isOutput=False)
    aw_d = nc.declare_dram_parameter("alpha_w", [1, HID], F32, isOutput=False)
    out_d = nc.declare_dram_parameter("out", [4, NQUAD, NFQ, 512], BF16, isOutput=True)

    with tile.TileContext(nc) as tc:
        with tc.tile_pool(name="singles", bufs=1) as singles:
            wahT = singles.tile([128, NHC, BL], F32)  # WahT[h % 128, hc, b]
            awT = singles.tile([128, NHC], BF16)  # alpha_w^T chunks
            # exp(scores), 256-wide zero-padded slot per batch so the lo
            # alphaT transpose matmul can span a full 128 output partitions
            expS = singles.tile([1, BL, 256], BF16)
            nc.gpsimd.memset(expS[:], 0.0)
            sums = singles.tile([1, BL], F32)
            rsum = singles.tile([1, BL], F32)
            # 1/sum replicated 32-wide so the alphaT transpose matmuls can
            # produce alpha replicated across 32 columns (-> M=32 phase-2
            # weights that write every partition of their PSUM col group)
            rsum_rep = singles.tile([1, BL, 32], BF16)
            ones_row = singles.tile([1, 32], BF16)
            nc.gpsimd.memset(ones_row[:], 1.0)
            aT_sb = singles.tile([128, BL, 2, 32], BF16)  # alphaT cols (hi, lo)

            # Batch-loop SBUF pools are allocated FIRST so their zones never
            # overlap the setup pool's -- otherwise the first input DMAs
            # inherit released-zone deps on the setup computation.
            with (
                tc.tile_pool(name="hi", bufs=AF_BUFS) as pool_hi,
                tc.tile_pool(name="lo", bufs=AF_BUFS) as pool_lo,
                tc.tile_pool(name="pa", bufs=2) as pool_pa,
                tc.tile_pool(name="ea", bufs=2) as pool_ea,
                tc.tile_pool(name="e", bufs=2) as pool_e,
                tc.tile_pool(name="osb", bufs=2) as pool_osb,
            ):
                # ---------------- setup: weights ----------------
                with (
                    tc.tile_pool(name="setup_sb", bufs=1) as ssb,
                    tc.tile_pool(name="setup_ww", bufs=2) as sww,
                    tc.tile_pool(name="setup_ps", bufs=2, space=PSUM) as sps,
                    tc.tile_pool(name="setup_acc", bufs=1, space=PSUM) as sacc,
                ):
                    hT = ssb.tile([128, NRC, BL], BF16)
                    nc.sync.dma_start(
                        hT[:], h_d[:].rearrange("(rc p) b -> p rc b", p=128)
                    )
                    aw_sb = ssb.tile([1, HID], BF16)
                    # f32 -> bf16 cast during DMA is SWDGE-only
                    nc.gpsimd.dma_start(aw_sb[:], aw_d[:])
                    ones11 = ssb.tile([1, 1], BF16)
                    nc.gpsimd.memset(ones11[:], 1.0)

                    # alpha_w^T columns (bf16 to match bf16 e tiles)
                    for hc in range(NHC):
                        ps = sps.tile([128, 1], F32, tag="aw")
                        nc.tensor.matmul(
                            ps[:],
                            aw_sb[0:1, hc * 128 : (hc + 1) * 128],
                            ones11[:],
                            start=True,
                            stop=True,
                        )
                        nc.vector.tensor_copy(awT[:, hc : hc + 1], ps[:])

                    # WahT[h, b] = sum_r Wah_w[h, r] * h[b, r]; the Wah_w
                    # chunks stream through a small 2-buf pool
                    wahT_ps = [
                        sacc.tile([128, BL], F32, tag=f"acc{hc}", name=f"wahT_ps{hc}")
                        for hc in range(NHC)
                    ]
                    ww_r = ww_d[:].rearrange("(rc p) c -> p rc c", p=128)
                    for rc in range(NRC):
                        wwc = sww.tile([128, HID], BF16, tag="ww")
                        nc.sync.dma_start(wwc[:], ww_r[:, rc])
                        for hc in range(NHC):
                            nc.tensor.matmul(
                                wahT_ps[hc][:],
                                wwc[:, hc * 128 : (hc + 1) * 128],
                                hT[:, rc, :],
                                start=(rc == 0),
                                stop=(rc == NRC - 1),
                            )
                    for hc in range(NHC):
                        nc.vector.tensor_copy(wahT[:, hc, :], wahT_ps[hc][:])

                # ---------------- streaming batch loop ----------------
                with (
                    tc.tile_pool(name="sc_ps", bufs=2, space=PSUM) as pool_sc,
                    tc.tile_pool(name="aT_ps", bufs=2, space=PSUM) as pool_aT,
                    tc.tile_pool(name="ao_ps", bufs=1, space=PSUM) as pool_ao,
                ):
                    def phase2(oct_, af_hi, af_lo, osb):
                        for r in range(2):
                            quad = 2 * oct_ + r
                            for q in range(NFQ):
                                ao = pool_ao.tile([128, 512], F32, tag=f"q{q}")
                                fsl = slice(q * 512, (q + 1) * 512)
                                for j in range(4):
                                    nc.tensor.matmul(
                                        ao[32 * j : 32 * j + 32, :],
                                        aT_sb[:, 4 * quad + j, 0],
                                        af_hi[:, 4 * r + j, fsl],
                                        start=True,
                                        stop=False,
                                        tile_position=(0, 32 * j),
                                    )
                                    nc.tensor.matmul(
                                        ao[32 * j : 32 * j + 32, :],
                                        aT_sb[0:L_LO, 4 * quad + j, 1],
                                        af_lo[0:L_LO, 4 * r + j, fsl],
                                        start=False,
                                        stop=True,
                                        tile_position=(0, 32 * j),
                                    )
                                # full-width copy: partition-strided APs are
                                # illegal on compute engines; copying all
                                # 128 lanes costs the same (per-lane elems).
                                # Split across DVE and ACT to balance load.
                                if q % 2 == 0:
                                    nc.vector.tensor_copy(osb[:, r, q, :], ao[:])
                                else:
                                    nc.scalar.copy(osb[:, r, q, :], ao[:])
                        # output DMAs: partition 32j carries quad-batch j
                        for j in range(4):
                            nc.sync.dma_start(
                                out_d[j : j + 1, 2 * oct_ : 2 * oct_ + 2],
                                osb[32 * j : 32 * j + 1],
                            )

                    # Software-pipelined: phase 2 of oct o-1 is emitted
                    # AFTER phase 1 of oct o, so the PE queue never
                    # head-of-line blocks on the af DMA of the current oct
                    # (that DMA was issued a full oct-period before phase2
                    # consumes it). Oct-sized af transfers give 32KB
                    # descriptors (16KB ones measured only ~15 GB/s per
                    # SDMA engine).
                    prev = None
                    for oct_ in range(NOCT):
                        af_hi = pool_hi.tile([128, 8, FEAT], BF16, tag="hi")
                        nc.sync.dma_start(af_hi[:], hi_d[oct_])
                        af_lo = pool_lo.tile([LPAD_LO, 8, FEAT], BF16, tag="lo")
                        nc.sync.dma_start(af_lo[:], lo_d[oct_])
                        pa_oct = pool_pa.tile([128, 4, NHC, 2, L], BF16, tag="pa")
                        nc.gpsimd.dma_start(pa_oct[:], pa_d[oct_])
                        osb = pool_osb.tile([128, 2, NFQ, 512], BF16, tag="osb")

                        # ---- phase 1 for the oct's four pairs ----
                        for pp in range(4):
                            p = 4 * oct_ + pp
                            b0 = 2 * p
                            # Wah broadcast-adds on DVE (265ns each true
                            # cost; GpSimd's generic path is 3.6us -- keep
                            # it off), then ONE tanh over the whole pair
                            # tile so ACT pays the 352-cycle instruction
                            # overhead once instead of 8 times
                            ea = pool_ea.tile([128, NHC, 2, L], BF16)
                            for hc in range(NHC):
                                for jb in range(2):
                                    b = b0 + jb
                                    nc.vector.tensor_scalar_add(
                                        ea[:, hc, jb, :],
                                        pa_oct[:, pp, hc, jb, :],
                                        wahT[:, hc, b : b + 1],
                                    )
                            e = pool_e.tile([128, NHC, 2, L], BF16)
                            nc.scalar.activation(e[:], ea[:], Tanh)

                            sc = pool_sc.tile([1, 2, L], F32)
                            for hc in range(NHC):
                                nc.tensor.matmul(
                                    sc[:],
                                    awT[:, hc : hc + 1],
                                    e[:, hc],
                                    start=(hc == 0),
                                    stop=(hc == NHC - 1),
                                )

                            for jb in range(2):
                                b = b0 + jb
                                nc.scalar.activation(
                                    expS[0:1, b, 0:L],
                                    sc[0:1, jb, :],
                                    Exp,
                                    accum_out=sums[0:1, b : b + 1],
                                )
                                nc.vector.reciprocal(
                                    rsum[0:1, b : b + 1], sums[0:1, b : b + 1]
                                )
                                nc.vector.tensor_scalar_mul(
                                    rsum_rep[0:1, b, :],
                                    ones_row[:],
                                    rsum[0:1, b : b + 1],
                                )
                                # alphaT columns via K=1 matmuls; rhs = the
                                # replicated 1/sum row, folding the softmax
                                # normalization in and replicating alpha to
                                # 32 columns
                                aT = pool_aT.tile([128, 2, 32], F32)
                                nc.tensor.matmul(
                                    aT[:, 0, :],
                                    expS[0:1, b, 0:128],
                                    rsum_rep[0:1, b, :],
                                    start=True,
                                    stop=True,
                                )
                                nc.tensor.matmul(
                                    aT[:, 1, :],
                                    expS[0:1, b, 128:256],
                                    rsum_rep[0:1, b, :],
                                    start=True,
                                    stop=True,
                                )
                                nc.vector.tensor_copy(aT_sb[:, b], aT[:])

                        # ---- phase 2 for the PREVIOUS quad ----
                        if prev is not None:
                            phase2(*prev)
                        prev = (oct_, af_hi, af_lo, osb)
                    phase2(*prev)

    if split:
        _split_sync(nc)
    return nc


_NC_CACHE = None


def _get_nc():
    global _NC_CACHE
    if _NC_CACHE is None:
        _NC_CACHE = build_nc()
    return _NC_CACHE


def _make_in_maps(h, att_feats, p_att_feats, Wah_w, alpha_w):
    import ml_dtypes

    bf = ml_dtypes.bfloat16
    h = np.ascontiguousarray(h, dtype=np.float32)
    att_feats = np.ascontiguousarray(att_feats, dtype=np.float32)
    p_att_feats = np.ascontiguousarray(p_att_feats, dtype=np.float32)
    wwT_host = np.ascontiguousarray(Wah_w.T).astype(bf)  # [RNN, HID]
    alpha_w = np.ascontiguousarray(alpha_w, dtype=np.float32)
    in_maps = []
    for i in range(NCORES):
        sl = slice(i * BL, (i + 1) * BL)
        # p_att: [BL, L, HID] -> [NOCT, 128, 4pairs, NHC, 2, L]
        pa = (
            p_att_feats[sl]
            .reshape(NOCT, 4, 2, L, NHC, 128)
            .transpose(0, 5, 1, 4, 2, 3)
            .astype(bf)
        )
        af = att_feats[sl].reshape(NOCT, 8, L, FEAT)
        af_hi = np.ascontiguousarray(af[:, :, :L_HI].transpose(0, 2, 1, 3)).astype(bf)
        af_lo = np.zeros((NOCT, LPAD_LO, 8, FEAT), dtype=bf)
        af_lo[:, :L_LO] = af[:, :, L_HI:].transpose(0, 2, 1, 3).astype(bf)
        in_maps.append(
            {
                "h": np.ascontiguousarray(h[sl].T).astype(bf),
                "p_att_feats": np.ascontiguousarray(pa),
                "att_hi": af_hi,
                "att_lo": af_lo,
                "Wah_w": wwT_host,
                "alpha_w": alpha_w,
            }
        )
    return in_maps


def _unpack_out(o):
    """[4, NQUAD, NFQ, 512] -> [BL, FEAT]"""
    return np.ascontiguousarray(
        np.asarray(o, dtype=np.float32).transpose(1, 0, 2, 3).reshape(BL, FEAT)
    )


def run_spmd(h, att_feats, p_att_feats, Wah_w, alpha_w, trace=False):
    """Run the SPMD kernel; returns (full_output, BassKernelResults)."""
    from concourse.bass_utils import run_bass_kernel_spmd

    nc = _get_nc()
    in_maps = _make_in_maps(h, att_feats, p_att_feats, Wah_w, alpha_w)
    res = run_bass_kernel_spmd(nc, in_maps, list(range(NCORES)), trace=trace)
    out = np.concatenate(
        [_unpack_out(res.results[i]["out"]) for i in range(NCORES)], axis=0
    )
    return out, res


def kernel(h, att_feats, p_att_feats, Wah_w, alpha_w):
    out, _ = run_spmd(h, att_feats, p_att_feats, Wah_w, alpha_w, trace=False)
    return out


# revision 43
# speedup vs baseline: 1.3864x; 1.1106x over previous
"""Trainium2 Bass kernel for additive-attention pooling (v2).

Computation (per batch row b):
    Wah   = h @ Wah_w.T                         [B, HID]
    e     = tanh(Wah[:, None, :] + p_att_feats) [B, L, HID]
    s     = e @ alpha_w[0]                      [B, L]
    alpha = softmax(s, -1)                      [B, L]
    att   = sum_l alpha[b, l] * att_feats[b, l, :]   [B, FEAT]

Sharding: pure data parallel over the batch dim, 32 rows per core on 8
NeuronCores; the small Wah_w / alpha_w weights are replicated.

v2 dataflow changes vs the first working kernel (264 us):
  * p_att_feats arrives host-TRANSPOSED ([h%128 partitions, pair, hc,
    jb, l] bf16) so the kernel does ZERO PE transposes (the old kernel
    spent ~2/3 of its TensorE time on transpose matmuls and ran the PE
    at half clock from HAM oscillation).
  * The Wah broadcast-add moves off the ACT bias path onto DVE+GpSimd
    tensor_scalar adds (8 small adds per pair, engines alternated),
    followed by ONE tanh activation over the whole [128, 4*2*196] pair
    tile -- ACT instruction count drops 128 -> 16 for the tanh work.
  * Phase 2 (att = alpha^T @ att_feats) runs as 4-way column-tiled
    matmuls: batch j of a quad occupies PE column-group j
    (tile_position=(0,32j), alpha column as the stationary operand), so
    4 batches stream their att_feats concurrently and the PSUM output
    lands on partitions {0,32,64,96} -- the PSUM->SBUF copies are
    [4, 512] (4 active lanes) instead of [1, 512] (1 lane), which
    removes the ~80 us of single-lane copies the old kernel paid.
  * Outputs stage per quad and DMA out via strided-partition APs.

All data stays bf16 on the wire (fp8 was measured: rel_norm 2.7e-2 on
att_feats -- too close to the 2e-2 gate).

The walrus build in this image accepts only one semaphore wait and one
update per instruction; _split_sync() post-processes the scheduled BIR
to spread Tile's multi-wait/multi-update sync info onto NoOp carriers.
"""

import os
import sys
import types

sys.path.insert(0, "/opt/trn_rl_repo")

# This image's antenv package lacks axon_hooks; provide it so
# concourse.bass_utils can import it (trace path) without crashing.
if "antenv.axon_hooks" not in sys.modules:
    _m = types.ModuleType("antenv.axon_hooks")

    def _set_hook(h):
        _m._hook = h

    def _get_hook():
        return getattr(_m, "_hook", None)

    _m.set_axon_ntff_profile_hook = _set_hook
    _m.get_axon_ntff_profile_hook = _get_hook
    sys.modules["antenv.axon_hooks"] = _m
    import antenv

    antenv.axon_hooks = _m

import numpy as np  # noqa: E402
import bass_rust  # noqa: E402
import concourse.bass as bass  # noqa: E402
import concourse.tile as tile  # noqa: E402
from concourse import mybir  # noqa: E402

F32 = mybir.dt.float32
BF16 = mybir.dt.bfloat16
PSUM = bass.MemorySpace.PSUM
Tanh = mybir.ActivationFunctionType.Tanh
Exp = mybir.ActivationFunctionType.Exp

B, L, RNN, HID, FEAT = 256, 196, 1024, 512, 2048
NCORES = 8
BL = B // NCORES  # batch rows per core (32)
L_HI = 128
L_LO = L - L_HI  # 68
NHC = HID // 128  # 4 h chunks
NRC = RNN // 128  # 8 r chunks
NFQ = FEAT // 512  # 4 psum-bank-sized f chunks
NPAIR = BL // 2  # 16
NQUAD = BL // 4  # 8
NOCT = BL // 8  # 4
# lo rows padded 68 -> 80 (multiple of 16): the DMA fans across
# largest-divisor-of-desc-count<=16 engines, so 68 descriptors land on
# only 4 SDMA engines while 80 spread over 16
LPAD_LO = 80

AF_BUFS = int(os.environ.get("KERNEL_AF_BUFS", "2"))


def _split_sync(nc):
    """walrus in this image encodes at most ONE semaphore wait and ONE
    semaphore update per instruction; Tile freely emits several. Move the
    extras onto single-wait/single-update NoOp carriers on the same engine
    (engine queues are strict FIFO, so a preceding NoOp's wait gates the
    instruction and a following NoOp's update fires after it completes)."""
    dma_types = {
        "InstDMACopy",
        "InstTensorLoad",
        "InstTensorSave",
        "InstDmaTransposeAnt",
        "InstTensorCopy",
    }
    for f in nc.m.functions:
        for bb in f.blocks:
            new = []
            changed = False
            for ins in bb.instructions:
                si = ins.sync_info
                if si is None:
                    new.append(ins)
                    continue
                waits = list(si.on_wait)
                updates = list(si.on_update)
                if len(waits) <= 1 and len(updates) <= 1:
                    new.append(ins)
                    continue
                changed = True
                tname = type(ins).__name__
                for j, w in enumerate(waits[:-1]):
                    nop = mybir.InstNoOp(name=f"{ins.name}_w{j}", ins=[], outs=[])
                    nop.engine = ins.engine
                    nop.sync_info = bass_rust.SyncInfo(on_wait=[w], on_update=[])
                    new.append(nop)
                keep_w = waits[-1:]
                post_u = []
                keep_u = updates
                if len(updates) > 1:
                    if tname in dma_types:
                        raise RuntimeError(
                            f"DMA instruction {ins.name} carries {len(updates)} "
                            "sem updates; cannot split without changing semantics"
                        )
                    keep_u = updates[:1]
                    post_u = updates[1:]
                ins.sync_info = bass_rust.SyncInfo(on_wait=keep_w, on_update=keep_u)
                new.append(ins)
                for j, u in enumerate(post_u):
                    nop = mybir.InstNoOp(name=f"{ins.name}_u{j}", ins=[], outs=[])
                    nop.engine = ins.engine
                    nop.sync_info = bass_rust.SyncInfo(on_wait=[], on_update=[u])
                    new.append(nop)
            if changed:
                bb.instructions = new


def build_nc(split=True):
    """Inputs arrive host-packed (see _make_in_maps):
      h:       [RNN, BL]                 bf16  (r-major)
      Wah_w:   [RNN, HID]                bf16  (r-major)
      alpha_w: [1, HID]                  f32
      p_att_feats: [128, NPAIR, NHC, 2, L]     bf16 (h%128 on partitions)
      att_hi:  [NQUAD, 128, 4, FEAT]     bf16  (l rows 0..127)
      att_lo:  [NQUAD, L_LO, 4, FEAT]    bf16  (l rows 128..195)
    Output:
      out:     [4, NQUAD, NFQ, 512]      f32   (att[4*qd+j, 512*q+x] =
                                                out[j, qd, q, x])
    """
    nc = bass.Bass()
    h_d = nc.declare_dram_parameter("h", [RNN, BL], BF16, isOutput=False)
    pa_d = nc.declare_dram_parameter(
        "p_att_feats", [NOCT, 128, 4, NHC, 2, L], BF16, isOutput=False
    )
    hi_d = nc.declare_dram_parameter(
        "att_hi", [NOCT, 128, 8, FEAT], BF16, isOutput=False
    )
    lo_d = nc.declare_dram_parameter(
        "att_lo", [NOCT, LPAD_LO, 8, FEAT], BF16, isOutput=False
    )
    ww_d = nc.declare_dram_parameter("Wah_w", [RNN, HID], BF16, isOutput=False)
    aw_d = nc.declare_dram_parameter("alpha_w", [1, HID], F32, isOutput=False)
    out_d = nc.declare_dram_parameter("out", [4, NQUAD, NFQ, 512], BF16, isOutput=True)

    with tile.TileContext(nc) as tc:
        with tc.tile_pool(name="singles", bufs=1) as singles:
            wahT = singles.tile([128, NHC, BL], F32)  # WahT[h % 128, hc, b]
            awT = singles.tile([128, NHC], BF16)  # alpha_w^T chunks
            # exp(scores), 256-wide zero-padded slot per batch so the lo
            # alphaT transpose matmul can span a full 128 output partitions
            expS = singles.tile([1, BL, 256], BF16)
            nc.gpsimd.memset(expS[:], 0.0)
            sums = singles.tile([1, BL], F32)
            rsum = singles.tile([1, BL], F32)
            # 1/sum replicated 32-wide so the alphaT transpose matmuls can
            # produce alpha replicated across 32 columns (-> M=32 phase-2
            # weights that write every partition of their PSUM col group)
            rsum_rep = singles.tile([1, BL, 32], BF16)
            ones_row = singles.tile([1, 32], BF16)
            nc.gpsimd.memset(ones_row[:], 1.0)
            aT_sb = singles.tile([128, BL, 2, 32], BF16)  # alphaT cols (hi, lo)

            # Batch-loop SBUF pools are allocated FIRST so their zones never
            # overlap the setup pool's -- otherwise the first input DMAs
            # inherit released-zone deps on the setup computation.
            with (
                tc.tile_pool(name="hi", bufs=AF_BUFS) as pool_hi,
                tc.tile_pool(name="lo", bufs=AF_BUFS) as pool_lo,
                tc.tile_pool(name="pa", bufs=2) as pool_pa,
                tc.tile_pool(name="ea", bufs=2) as pool_ea,
                tc.tile_pool(name="e", bufs=2) as pool_e,
                tc.tile_pool(name="osb", bufs=2) as pool_osb,
            ):
                # ---------------- setup: weights ----------------
                with (
                    tc.tile_pool(name="setup_sb", bufs=1) as ssb,
                    tc.tile_pool(name="setup_ww", bufs=2) as sww,
                    tc.tile_pool(name="setup_ps", bufs=2, space=PSUM) as sps,
                    tc.tile_pool(name="setup_acc", bufs=1, space=PSUM) as sacc,
                ):
                    # setup DMAs ride the SWDGE queue so the sync HWDGE
                    # ring carries nothing but the att_feats stream
                    hT = ssb.tile([128, NRC, BL], BF16)
                    nc.gpsimd.dma_start(
                        hT[:], h_d[:].rearrange("(rc p) b -> p rc b", p=128)
                    )
                    aw_sb = ssb.tile([1, HID], BF16)
                    # f32 -> bf16 cast during DMA is SWDGE-only
                    nc.gpsimd.dma_start(aw_sb[:], aw_d[:])
                    ones11 = ssb.tile([1, 1], BF16)
                    nc.gpsimd.memset(ones11[:], 1.0)

                    # alpha_w^T columns (bf16 to match bf16 e tiles)
                    for hc in range(NHC):
                        ps = sps.tile([128, 1], F32, tag="aw")
                        nc.tensor.matmul(
                            ps[:],
                            aw_sb[0:1, hc * 128 : (hc + 1) * 128],
                            ones11[:],
                            start=True,
                            stop=True,
                        )
                        nc.vector.tensor_copy(awT[:, hc : hc + 1], ps[:])

                    # WahT[h, b] = sum_r Wah_w[h, r] * h[b, r]; the Wah_w
                    # chunks stream through a small 2-buf pool
                    wahT_ps = [
                        sacc.tile([128, BL], F32, tag=f"acc{hc}", name=f"wahT_ps{hc}")
                        for hc in range(NHC)
                    ]
                    ww_r = ww_d[:].rearrange("(rc p) c -> p rc c", p=128)
                    for rc in range(NRC):
                        wwc = sww.tile([128, HID], BF16, tag="ww")
                        nc.gpsimd.dma_start(wwc[:], ww_r[:, rc])
                        for hc in range(NHC):
                            nc.tensor.matmul(
                                wahT_ps[hc][:],
                                wwc[:, hc * 128 : (hc + 1) * 128],
                                hT[:, rc, :],
                                start=(rc == 0),
                                stop=(rc == NRC - 1),
                            )
                    for hc in range(NHC):
                        nc.vector.tensor_copy(wahT[:, hc, :], wahT_ps[hc][:])

                # ---------------- streaming batch loop ----------------
                with (
                    tc.tile_pool(name="sc_ps", bufs=2, space=PSUM) as pool_sc,
                    tc.tile_pool(name="aT_ps", bufs=2, space=PSUM) as pool_aT,
                    tc.tile_pool(name="ao_ps", bufs=1, space=PSUM) as pool_ao,
                ):
                    def phase2(oct_, af_hi, af_lo, osb):
                        for r in range(2):
                            quad = 2 * oct_ + r
                            for q in range(NFQ):
                                ao = pool_ao.tile([128, 512], F32, tag=f"q{q}")
                                fsl = slice(q * 512, (q + 1) * 512)
                                for j in range(4):
                                    nc.tensor.matmul(
                                        ao[32 * j : 32 * j + 32, :],
                                        aT_sb[:, 4 * quad + j, 0],
                                        af_hi[:, 4 * r + j, fsl],
                                        start=True,
                                        stop=False,
                                        tile_position=(0, 32 * j),
                                    )
                                    nc.tensor.matmul(
                                        ao[32 * j : 32 * j + 32, :],
                                        aT_sb[0:L_LO, 4 * quad + j, 1],
                                        af_lo[0:L_LO, 4 * r + j, fsl],
                                        start=False,
                                        stop=True,
                                        tile_position=(0, 32 * j),
                                    )
                                # full-width copy: partition-strided APs are
                                # illegal on compute engines; copying all
                                # 128 lanes costs the same (per-lane elems).
                                # Split across DVE and ACT to balance load.
                                if q % 2 == 0:
                                    nc.vector.tensor_copy(osb[:, r, q, :], ao[:])
                                else:
                                    nc.scalar.copy(osb[:, r, q, :], ao[:])
                        # output DMAs: partition 32j carries quad-batch j
                        for j in range(4):
                            nc.sync.dma_start(
                                out_d[j : j + 1, 2 * oct_ : 2 * oct_ + 2],
                                osb[32 * j : 32 * j + 1],
                            )

                    # Software-pipelined: phase 2 of oct o-1 is emitted
                    # AFTER phase 1 of oct o, so the PE queue never
                    # head-of-line blocks on the af DMA of the current oct
                    # (that DMA was issued a full oct-period before phase2
                    # consumes it). Oct-sized af transfers give 32KB
                    # descriptors (16KB ones measured only ~15 GB/s per
                    # SDMA engine).
                    prev = None
                    for oct_ in range(NOCT):
                        af_hi = pool_hi.tile([128, 8, FEAT], BF16, tag="hi")
                        nc.sync.dma_start(af_hi[:], hi_d[oct_])
                        af_lo = pool_lo.tile([LPAD_LO, 8, FEAT], BF16, tag="lo")
                        nc.sync.dma_start(af_lo[:], lo_d[oct_])
                        pa_oct = pool_pa.tile([128, 4, NHC, 2, L], BF16, tag="pa")
                        nc.gpsimd.dma_start(pa_oct[:], pa_d[oct_])
                        osb = pool_osb.tile([128, 2, NFQ, 512], BF16, tag="osb")

                        # ---- phase 1 for the oct's four pairs ----
                        for pp in range(4):
                            p = 4 * oct_ + pp
                            b0 = 2 * p
                            # Wah broadcast-adds on DVE (265ns each true
                            # cost; GpSimd's generic path is 3.6us -- keep
                            # it off), then ONE tanh over the whole pair
                            # tile so ACT pays the 352-cycle instruction
                            # overhead once instead of 8 times
                            ea = pool_ea.tile([128, NHC, 2, L], BF16)
                            for hc in range(NHC):
                                for jb in range(2):
                                    b = b0 + jb
                                    nc.vector.tensor_scalar_add(
                                        ea[:, hc, jb, :],
                                        pa_oct[:, pp, hc, jb, :],
                                        wahT[:, hc, b : b + 1],
                                    )
                            e = pool_e.tile([128, NHC, 2, L], BF16)
                            nc.scalar.activation(e[:], ea[:], Tanh)

                            sc = pool_sc.tile([1, 2, L], F32)
                            for hc in range(NHC):
                                nc.tensor.matmul(
                                    sc[:],
                                    awT[:, hc : hc + 1],
                                    e[:, hc],
                                    start=(hc == 0),
                                    stop=(hc == NHC - 1),
                                )

                            for jb in range(2):
                                b = b0 + jb
                                nc.scalar.activation(
                                    expS[0:1, b, 0:L],
                                    sc[0:1, jb, :],
                                    Exp,
                                    accum_out=sums[0:1, b : b + 1],
                                )
                                nc.vector.reciprocal(
                                    rsum[0:1, b : b + 1], sums[0:1, b : b + 1]
                                )
                                nc.vector.tensor_scalar_mul(
                                    rsum_rep[0:1, b, :],
                                    ones_row[:],
                                    rsum[0:1, b : b + 1],
                                )
                                # alphaT columns via K=1 matmuls; rhs = the
                                # replicated 1/sum row, folding the softmax
                                # normalization in and replicating alpha to
                                # 32 columns
                                aT = pool_aT.tile([128, 2, 32], F32)
                                nc.tensor.matmul(
                                    aT[:, 0, :],
                                    expS[0:1, b, 0:128],
                                    rsum_rep[0:1, b, :],
                                    start=True,
                                    stop=True,
                                )
                                nc.tensor.matmul(
                                    aT[:, 1, :],
                                    expS[0:1, b, 128:256],
                                    rsum_rep[0:1, b, :],
                                    start=True,
                                    stop=True,
                                )
                                nc.vector.tensor_copy(aT_sb[:, b], aT[:])

                        # ---- phase 2 for the PREVIOUS quad ----
                        if prev is not None:
                            phase2(*prev)
                        prev = (oct_, af_hi, af_lo, osb)
                    phase2(*prev)

    if split:
        _split_sync(nc)
    return nc


_NC_CACHE = None


def _get_nc():
    global _NC_CACHE
    if _NC_CACHE is None:
        _NC_CACHE = build_nc()
    return _NC_CACHE


def _make_in_maps(h, att_feats, p_att_feats, Wah_w, alpha_w):
    import ml_dtypes

    bf = ml_dtypes.bfloat16
    h = np.ascontiguousarray(h, dtype=np.float32)
    att_feats = np.ascontiguousarray(att_feats, dtype=np.float32)
    p_att_feats = np.ascontiguousarray(p_att_feats, dtype=np.float32)
    wwT_host = np.ascontiguousarray(Wah_w.T).astype(bf)  # [RNN, HID]
    alpha_w = np.ascontiguousarray(alpha_w, dtype=np.float32)
    in_maps = []
    for i in range(NCORES):
        sl = slice(i * BL, (i + 1) * BL)
        # p_att: [BL, L, HID] -> [NOCT, 128, 4pairs, NHC, 2, L]
        pa = (
            p_att_feats[sl]
            .reshape(NOCT, 4, 2, L, NHC, 128)
            .transpose(0, 5, 1, 4, 2, 3)
            .astype(bf)
        )
        af = att_feats[sl].reshape(NOCT, 8, L, FEAT)
        af_hi = np.ascontiguousarray(af[:, :, :L_HI].transpose(0, 2, 1, 3)).astype(bf)
        af_lo = np.zeros((NOCT, LPAD_LO, 8, FEAT), dtype=bf)
        af_lo[:, :L_LO] = af[:, :, L_HI:].transpose(0, 2, 1, 3).astype(bf)
        in_maps.append(
            {
                "h": np.ascontiguousarray(h[sl].T).astype(bf),
                "p_att_feats": np.ascontiguousarray(pa),
                "att_hi": af_hi,
                "att_lo": af_lo,
                "Wah_w": wwT_host,
                "alpha_w": alpha_w,
            }
        )
    return in_maps


def _unpack_out(o):
    """[4, NQUAD, NFQ, 512] -> [BL, FEAT]"""
    return np.ascontiguousarray(
        np.asarray(o, dtype=np.float32).transpose(1, 0, 2, 3).reshape(BL, FEAT)
    )


def run_spmd(h, att_feats, p_att_feats, Wah_w, alpha_w, trace=False):
    """Run the SPMD kernel; returns (full_output, BassKernelResults)."""
    from concourse.bass_utils import run_bass_kernel_spmd

    nc = _get_nc()
    in_maps = _make_in_maps(h, att_feats, p_att_feats, Wah_w, alpha_w)
    res = run_bass_kernel_spmd(nc, in_maps, list(range(NCORES)), trace=trace)
    out = np.concatenate(
        [_unpack_out(res.results[i]["out"]) for i in range(NCORES)], axis=0
    )
    return out, res


def kernel(h, att_feats, p_att_feats, Wah_w, alpha_w):
    out, _ = run_spmd(h, att_feats, p_att_feats, Wah_w, alpha_w, trace=False)
    return out
